# revision 7
# baseline (speedup 1.0000x reference)
"""Trainium2 Bass kernel: 16-head MHA (B=4, S=2048, E=1024, Dh=64), 8 cores.

Sharding: core c handles batch b = c//2 and head-group g = c%2 (8 heads).
Each core computes its 8 heads' attention plus the partial output
projection in transposed layout oT[e, s]; the host sums the two
head-group partials per batch, transposes, and adds bo.

Per-core dataflow (all matmuls bf16 with fp32 PSUM accumulation):
  qT/kT[d, s]  = Wq/Wk.T @ xT          (per head-pair, d stacked 2x64)
  v[t, hd]     = xT.T @ Wv + ones.T@bv (natural layout, + ones col for colsum)
  scoresT[t,s] = kT.T @ qT   (row-tiled: 2 heads in rows 0-63 / 64-127)
  expT         = exp(0.125 * scoresT)  (ScalarE, cast to bf16)
  zT_un[d,s],colsum[s] = v_aug.T @ expT  (M=65: row 64 = colsum)
  zT           = zT_un * bcast(1/colsum) (bcast via K=1 matmul)
  oT[e, s]    += Wo_h.T @ zT_h  (accumulated over the core's 8 heads)
"""

import numpy as np
import ml_dtypes

B, S, E = 4, 2048, 1024
H, Dh = 16, 64
N_CORES = 8
HPC = 8          # heads per core
MP = 4           # head-pairs per core
SC, SCW = 4, 512  # s-chunks
TC, TCW = 16, 128  # t-chunks
KE = 8           # k-tiles over E
ECN = 8          # e-chunks of 128 (outT partition tiles)

BF16 = ml_dtypes.bfloat16

_PROG = None


def _build_program(repeats=None):
    """Emit the Bass/Tile program. Returns (nc, names_dict).

    repeats: if set, wrap the whole body in a For_i loop (for marginal
    per-iteration HW timing; not used by the graded kernel() path).
    """
    from contextlib import ExitStack

    import concourse.mybir as mybir
    import concourse.tile as tile
    from concourse import bacc

    dt = mybir.dt
    AF = mybir.ActivationFunctionType
    OP = mybir.AluOpType

    nc = bacc.Bacc(None, target_bir_lowering=False, debug=False)
    with tile.TileContext(nc) as tc:
        with tc.tile_pool(name="dram", bufs=1, space="DRAM") as dram:
            xT_d = dram.tile([E, S], dt.bfloat16, kind="ExternalInput")
            wq_d = dram.tile([E, HPC * Dh], dt.bfloat16, kind="ExternalInput")
            wk_d = dram.tile([E, HPC * Dh], dt.bfloat16, kind="ExternalInput")
            wv_d = dram.tile([E, HPC * Dh], dt.bfloat16, kind="ExternalInput")
            wo_d = dram.tile([Dh, HPC, E], dt.bfloat16, kind="ExternalInput")
            bq_d = dram.tile([128, MP], dt.float32, kind="ExternalInput")
            bk_d = dram.tile([128, MP], dt.float32, kind="ExternalInput")
            bv_d = dram.tile([1, HPC * Dh], dt.bfloat16, kind="ExternalInput")
            oT_d = dram.tile([E, S], dt.float32, kind="ExternalOutput")

            with (
                tc.tile_pool(name="const", bufs=1) as const,
                tc.tile_pool(name="work", bufs=2) as work,
                tc.tile_pool(name="psum", bufs=1, space="PSUM") as psum,
                tc.tile_pool(name="psum_sT", bufs=2, space="PSUM") as psum_sT,
                tc.tile_pool(name="psum_z", bufs=3, space="PSUM") as psum_z,
                ExitStack() as _es,
            ):
                if repeats is not None:
                    _es.enter_context(tc.For_i(
                        0, repeats, 1,
                        hint_engines=(
                            mybir.EngineType.PE, mybir.EngineType.Activation,
                            mybir.EngineType.DVE, mybir.EngineType.SP,
                            mybir.EngineType.Pool,
                        ),
                    ))
                # ---- persistent SBUF ----
                xT = const.tile([128, KE, S], dt.bfloat16)
                wq = const.tile([128, KE, HPC * Dh], dt.bfloat16)
                wk = const.tile([128, KE, HPC * Dh], dt.bfloat16)
                wv = const.tile([128, KE, HPC * Dh], dt.bfloat16)
                wo = const.tile([Dh, HPC, E], dt.bfloat16)
                bq = const.tile([128, MP], dt.float32)
                bk = const.tile([128, MP], dt.float32)
                bv = const.tile([1, HPC * Dh], dt.bfloat16)
                ones1 = const.tile([1, 128], dt.bfloat16)
                qT2 = const.tile([128, MP, S], dt.bfloat16)
                kT2 = const.tile([128, MP, S], dt.bfloat16)
                v_sb = const.tile([128, TC, HPC, Dh + 1], dt.bfloat16)

                # ---- input DMAs ----
                nc.sync.dma_start(xT[:, :, :], xT_d[:].rearrange("(a p) c -> p a c", p=128))
                nc.sync.dma_start(wq[:, :, :], wq_d[:].rearrange("(a p) c -> p a c", p=128))
                nc.sync.dma_start(wk[:, :, :], wk_d[:].rearrange("(a p) c -> p a c", p=128))
                nc.sync.dma_start(wv[:, :, :], wv_d[:].rearrange("(a p) c -> p a c", p=128))
                nc.sync.dma_start(wo[:, :, :], wo_d[:])
                nc.sync.dma_start(bq[:, :], bq_d[:])
                nc.sync.dma_start(bk[:, :], bk_d[:])
                nc.sync.dma_start(bv[:, :], bv_d[:])
                nc.vector.memset(ones1[:, :], 1.0)
                nc.vector.memset(v_sb[:, :, :, :], 1.0)

                # ---- projections: qT2 / kT2 (layout [d(2 heads), s]) ----
                for m in range(MP):
                    for sc in range(SC):
                        ssl = slice(sc * SCW, (sc + 1) * SCW)
                        for w_sb, b_sb, dst in ((wq, bq, qT2), (wk, bk, kT2)):
                            p = psum.tile([128, SCW], dt.float32, tag="big")
                            for k in range(KE):
                                nc.tensor.matmul(
                                    p[:, :],
                                    w_sb[:, k, m * 128:(m + 1) * 128],
                                    xT[:, k, ssl],
                                    start=(k == 0), stop=(k == KE - 1),
                                )
                            nc.vector.tensor_scalar_add(
                                dst[:, m, ssl], p[:, :], b_sb[:, m:m + 1]
                            )

                # ---- projection: v (natural layout [t, hd] + bias + ones col) ----
                for t in range(TC):
                    tsl = slice(t * TCW, (t + 1) * TCW)
                    p = psum.tile([128, HPC * Dh], dt.float32, tag="big")
                    for k in range(KE):
                        nc.tensor.matmul(
                            p[:, :], xT[:, k, tsl], wv[:, k, :],
                            start=(k == 0), stop=False,
                        )
                    nc.tensor.matmul(
                        p[:, :], ones1[0:1, :], bv[0:1, :], start=False, stop=True,
                    )
                    nc.vector.tensor_copy(
                        v_sb[:, t, :, 0:Dh],
                        p[:, :].rearrange("p (h c) -> p h c", c=Dh),
                    )

                # ---- attention + output projection, per s-chunk ----
                for sc in range(SC):
                    ssl = slice(sc * SCW, (sc + 1) * SCW)
                    zT = work.tile([Dh, HPC, SCW], dt.bfloat16, tag="zT")
                    for h in range(HPC):
                        m, hh = divmod(h, 2)
                        hoff = hh * Dh
                        eT = work.tile([128, TC, SCW], dt.bfloat16, tag="expT")
                        for t2 in range(TC // 2):
                            pst = psum_sT.tile([128, 2 * SCW], dt.float32, tag="sT")
                            for j in range(2):
                                t = 2 * t2 + j
                                nc.tensor.matmul(
                                    pst[:, j * SCW:(j + 1) * SCW],
                                    kT2[hoff:hoff + Dh, m, t * TCW:(t + 1) * TCW],
                                    qT2[hoff:hoff + Dh, m, ssl],
                                    start=True, stop=True,
                                    tile_position=(hoff, 0),
                                )
                            nc.scalar.activation(
                                eT[:, 2 * t2:2 * t2 + 2, :], pst[:, :],
                                AF.Exp, scale=0.125,
                            )
                        pz = psum_z.tile([Dh + 1, SCW], dt.float32, tag="z")
                        for t in range(TC):
                            nc.tensor.matmul(
                                pz[:, :], v_sb[:, t, h, :], eT[:, t, :],
                                start=(t == 0), stop=(t == TC - 1),
                            )
                        cs = work.tile([1, SCW], dt.bfloat16, tag="cs")
                        nc.vector.tensor_copy(cs[0:1, :], pz[Dh:Dh + 1, :])
                        pbc = psum_z.tile([Dh, SCW], dt.float32, tag="z")
                        nc.tensor.matmul(
                            pbc[:, :], ones1[0:1, 0:Dh], cs[0:1, :],
                            start=True, stop=True,
                        )
                        bch = work.tile([Dh, SCW], dt.float32, tag="bch")
                        nc.vector.reciprocal(bch[:, :], pbc[:, :])
                        nc.vector.tensor_tensor(
                            zT[:, h, :], pz[0:Dh, :], bch[:, :], OP.mult
                        )
                    for ec in range(ECN):
                        po = psum.tile([128, SCW], dt.float32, tag="big")
                        for h in range(HPC):
                            nc.tensor.matmul(
                                po[:, :],
                                wo[:, h, ec * 128:(ec + 1) * 128],
                                zT[:, h, :],
                                start=(h == 0), stop=(h == HPC - 1),
                            )
                        ob = work.tile([128, SCW], dt.float32, tag="ob")
                        nc.vector.tensor_copy(ob[:, :], po[:, :])
                        nc.sync.dma_start(
                            oT_d[ec * 128:(ec + 1) * 128, ssl], ob[:, :]
                        )

    nc.compile()
    names = {
        "xT": xT_d.name, "wq": wq_d.name, "wk": wk_d.name, "wv": wv_d.name,
        "wo": wo_d.name, "bq": bq_d.name, "bk": bk_d.name, "bv": bv_d.name,
        "oT": oT_d.name,
    }
    return nc, names


def get_program():
    global _PROG
    if _PROG is None:
        _PROG = _build_program()
    return _PROG


def make_in_maps(x, Wq, bq, Wk, bk, Wv, bv, Wo, names):
    """Host-side sharding: per-core input dict (bf16 casts + layout prep)."""
    in_maps = []
    for c in range(N_CORES):
        b, g = divmod(c, 2)
        hsl = slice(g * HPC, (g + 1) * HPC)
        xT_c = np.ascontiguousarray(x[b].T).astype(BF16)                 # [E, S]
        wq_c = np.ascontiguousarray(
            Wq[hsl].transpose(1, 0, 2).reshape(E, HPC * Dh)).astype(BF16)
        wk_c = np.ascontiguousarray(
            Wk[hsl].transpose(1, 0, 2).reshape(E, HPC * Dh)).astype(BF16)
        wv_c = np.ascontiguousarray(
            Wv[hsl].transpose(1, 0, 2).reshape(E, HPC * Dh)).astype(BF16)
        # Wo rows for this head group, packed [Dh, HPC, E] (head on free axis)
        wo_c = np.ascontiguousarray(
            Wo[g * HPC * Dh:(g + 1) * HPC * Dh].reshape(HPC, Dh, E)
            .transpose(1, 0, 2)).astype(BF16)
        bq_c = np.ascontiguousarray(bq[hsl].reshape(MP, 128).T).astype(np.float32)
        bk_c = np.ascontiguousarray(bk[hsl].reshape(MP, 128).T).astype(np.float32)
        bv_c = bv[hsl].reshape(1, HPC * Dh).astype(BF16)
        in_maps.append({
            names["xT"]: xT_c, names["wq"]: wq_c, names["wk"]: wk_c,
            names["wv"]: wv_c, names["wo"]: wo_c, names["bq"]: bq_c,
            names["bk"]: bk_c, names["bv"]: bv_c,
        })
    return in_maps


def combine_outputs(results, bo, names):
    """Host-side unshard: sum head-group partials, transpose, add bo."""
    out = np.empty((B, S, E), np.float32)
    for b in range(B):
        oT = results[2 * b][names["oT"]] + results[2 * b + 1][names["oT"]]
        out[b] = oT.T + bo
    return out


def kernel(x, Wq, bq, Wk, bk, Wv, bv, Wo, bo):
    from concourse.bass_utils import run_bass_kernel_spmd

    nc, names = get_program()
    in_maps = make_in_maps(
        np.asarray(x), np.asarray(Wq), np.asarray(bq), np.asarray(Wk),
        np.asarray(bk), np.asarray(Wv), np.asarray(bv), np.asarray(Wo), names,
    )
    res = run_bass_kernel_spmd(nc, in_maps, core_ids=list(range(N_CORES)))
    return combine_outputs(res.results, np.asarray(bo, np.float32), names)


# revision 10
# speedup vs baseline: 5.3011x; 5.3011x over previous
"""Trainium2 Bass kernel: 16-head MHA (B=4, S=2048, E=1024, Dh=64), 8 cores.

Sharding: core c handles batch b = c//2 and head-group g = c%2 (8 heads).
Each core computes its 8 heads' attention plus the partial output
projection in transposed layout oT[e, s]; the host sums the two
head-group partials per batch, transposes, and adds bo.

Per-core dataflow (all matmuls bf16 with fp32 PSUM accumulation):
  qT/kT[d, s]  = Wq/Wk.T @ xT          (per head-pair, d stacked 2x64)
  v[t, hd]     = xT.T @ Wv + ones.T@bv (natural layout, + ones col for colsum)
  scoresT[t,s] = kT.T @ qT   (row-tiled: 2 heads in rows 0-63 / 64-127)
  expT         = exp(0.125 * scoresT)  (ScalarE, cast to bf16)
  zT_un[d,s],colsum[s] = v_aug.T @ expT  (M=65: row 64 = colsum)
  zT           = zT_un * bcast(1/colsum) (bcast via K=1 matmul)
  oT[e, s]    += Wo_h.T @ zT_h  (accumulated over the core's 8 heads)
"""

import numpy as np
import ml_dtypes

B, S, E = 4, 2048, 1024
H, Dh = 16, 64
N_CORES = 8
HPC = 8          # heads per core
MP = 4           # head-pairs per core
SC, SCW = 4, 512  # s-chunks
TC, TCW = 16, 128  # t-chunks
KE = 8           # k-tiles over E
ECN = 8          # e-chunks of 128 (outT partition tiles)

BF16 = ml_dtypes.bfloat16

_PROG = None


def _build_program(repeats=None, timing=False):
    """Emit the Bass/Tile program. Returns (nc, names_dict).

    repeats: if set, wrap the whole body in a For_i loop (for marginal
    per-iteration HW timing; not used by the graded kernel() path).
    timing: demote the real output to internal DRAM and expose a tiny
    dummy output instead, so timing calls don't pay output transfers.
    """
    from contextlib import ExitStack

    import concourse.mybir as mybir
    import concourse.tile as tile
    from concourse import bacc

    dt = mybir.dt
    AF = mybir.ActivationFunctionType
    OP = mybir.AluOpType

    nc = bacc.Bacc(None, target_bir_lowering=False, debug=False)
    with tile.TileContext(nc) as tc:
        with tc.tile_pool(name="dram", bufs=1, space="DRAM") as dram:
            xT_d = dram.tile([E, S], dt.bfloat16, kind="ExternalInput")
            wq_d = dram.tile([E, HPC * Dh], dt.bfloat16, kind="ExternalInput")
            wk_d = dram.tile([E, HPC * Dh], dt.bfloat16, kind="ExternalInput")
            wv_d = dram.tile([E, HPC * Dh], dt.bfloat16, kind="ExternalInput")
            wo_d = dram.tile([Dh, HPC, E], dt.bfloat16, kind="ExternalInput")
            bq_d = dram.tile([128, MP], dt.float32, kind="ExternalInput")
            bk_d = dram.tile([128, MP], dt.float32, kind="ExternalInput")
            bv_d = dram.tile([1, HPC * Dh], dt.bfloat16, kind="ExternalInput")
            if timing:
                oT_d = dram.tile([E, S], dt.float32, kind="Internal")
                dummy_d = dram.tile([1, 4], dt.float32, kind="ExternalOutput")
            else:
                oT_d = dram.tile([E, S], dt.float32, kind="ExternalOutput")
                dummy_d = None

            with (
                tc.tile_pool(name="const", bufs=1) as const,
                tc.tile_pool(name="work", bufs=2) as work,
                tc.tile_pool(name="psum", bufs=1, space="PSUM") as psum,
                tc.tile_pool(name="psum_sT", bufs=2, space="PSUM") as psum_sT,
                tc.tile_pool(name="psum_z", bufs=3, space="PSUM") as psum_z,
                ExitStack() as _es,
            ):
                if repeats is not None:
                    _es.enter_context(tc.For_i(
                        0, repeats, 1,
                        hint_engines=(
                            mybir.EngineType.PE, mybir.EngineType.Activation,
                            mybir.EngineType.DVE, mybir.EngineType.SP,
                            mybir.EngineType.Pool,
                        ),
                    ))
                # ---- persistent SBUF ----
                xT = const.tile([128, KE, S], dt.bfloat16)
                wq = const.tile([128, KE, HPC * Dh], dt.bfloat16)
                wk = const.tile([128, KE, HPC * Dh], dt.bfloat16)
                wv = const.tile([128, KE, HPC * Dh], dt.bfloat16)
                wo = const.tile([Dh, HPC, E], dt.bfloat16)
                bq = const.tile([128, MP], dt.float32)
                bk = const.tile([128, MP], dt.float32)
                bv = const.tile([1, HPC * Dh], dt.bfloat16)
                ones1 = const.tile([1, 128], dt.bfloat16)
                qT2 = const.tile([128, MP, S], dt.bfloat16)
                kT2 = const.tile([128, MP, S], dt.bfloat16)
                v_sb = const.tile([128, TC, HPC, Dh + 1], dt.bfloat16)

                # ---- input DMAs ----
                nc.sync.dma_start(xT[:, :, :], xT_d[:].rearrange("(a p) c -> p a c", p=128))
                nc.sync.dma_start(wq[:, :, :], wq_d[:].rearrange("(a p) c -> p a c", p=128))
                nc.sync.dma_start(wk[:, :, :], wk_d[:].rearrange("(a p) c -> p a c", p=128))
                nc.sync.dma_start(wv[:, :, :], wv_d[:].rearrange("(a p) c -> p a c", p=128))
                nc.sync.dma_start(wo[:, :, :], wo_d[:])
                nc.sync.dma_start(bq[:, :], bq_d[:])
                nc.sync.dma_start(bk[:, :], bk_d[:])
                nc.sync.dma_start(bv[:, :], bv_d[:])
                nc.vector.memset(ones1[:, :], 1.0)
                nc.vector.memset(v_sb[:, :, :, :], 1.0)
                if dummy_d is not None:
                    dum = const.tile([1, 4], dt.float32)
                    nc.vector.memset(dum[:, :], 1.0)
                    nc.sync.dma_start(dummy_d[:, :], dum[:, :])

                # ---- projections: qT2 / kT2 (layout [d(2 heads), s]) ----
                for m in range(MP):
                    for sc in range(SC):
                        ssl = slice(sc * SCW, (sc + 1) * SCW)
                        for w_sb, b_sb, dst in ((wq, bq, qT2), (wk, bk, kT2)):
                            p = psum.tile([128, SCW], dt.float32, tag="big")
                            for k in range(KE):
                                nc.tensor.matmul(
                                    p[:, :],
                                    w_sb[:, k, m * 128:(m + 1) * 128],
                                    xT[:, k, ssl],
                                    start=(k == 0), stop=(k == KE - 1),
                                )
                            nc.vector.tensor_scalar_add(
                                dst[:, m, ssl], p[:, :], b_sb[:, m:m + 1]
                            )

                # ---- projection: v (natural layout [t, hd] + bias + ones col) ----
                for t in range(TC):
                    tsl = slice(t * TCW, (t + 1) * TCW)
                    p = psum.tile([128, HPC * Dh], dt.float32, tag="big")
                    for k in range(KE):
                        nc.tensor.matmul(
                            p[:, :], xT[:, k, tsl], wv[:, k, :],
                            start=(k == 0), stop=False,
                        )
                    nc.tensor.matmul(
                        p[:, :], ones1[0:1, :], bv[0:1, :], start=False, stop=True,
                    )
                    nc.vector.tensor_copy(
                        v_sb[:, t, :, 0:Dh],
                        p[:, :].rearrange("p (h c) -> p h c", c=Dh),
                    )

                # ---- attention + output projection, per s-chunk ----
                for sc in range(SC):
                    ssl = slice(sc * SCW, (sc + 1) * SCW)
                    zT = work.tile([Dh, HPC, SCW], dt.bfloat16, tag="zT")
                    for h in range(HPC):
                        m, hh = divmod(h, 2)
                        hoff = hh * Dh
                        eT = work.tile([128, TC, SCW], dt.bfloat16, tag="expT")
                        for t2 in range(TC // 2):
                            pst = psum_sT.tile([128, 2 * SCW], dt.float32, tag="sT")
                            for j in range(2):
                                t = 2 * t2 + j
                                nc.tensor.matmul(
                                    pst[:, j * SCW:(j + 1) * SCW],
                                    kT2[hoff:hoff + Dh, m, t * TCW:(t + 1) * TCW],
                                    qT2[hoff:hoff + Dh, m, ssl],
                                    start=True, stop=True,
                                    tile_position=(hoff, 0),
                                )
                            nc.scalar.activation(
                                eT[:, 2 * t2:2 * t2 + 2, :], pst[:, :],
                                AF.Exp, scale=0.125,
                            )
                        pz = psum_z.tile([Dh + 1, SCW], dt.float32, tag="z")
                        for t in range(TC):
                            nc.tensor.matmul(
                                pz[:, :], v_sb[:, t, h, :], eT[:, t, :],
                                start=(t == 0), stop=(t == TC - 1),
                            )
                        cs = work.tile([1, SCW], dt.bfloat16, tag="cs")
                        nc.vector.tensor_copy(cs[0:1, :], pz[Dh:Dh + 1, :])
                        pbc = psum_z.tile([Dh, SCW], dt.float32, tag="z")
                        nc.tensor.matmul(
                            pbc[:, :], ones1[0:1, 0:Dh], cs[0:1, :],
                            start=True, stop=True,
                        )
                        bch = work.tile([Dh, SCW], dt.float32, tag="bch")
                        nc.vector.reciprocal(bch[:, :], pbc[:, :])
                        nc.vector.tensor_tensor(
                            zT[:, h, :], pz[0:Dh, :], bch[:, :], OP.mult
                        )
                    for ec in range(ECN):
                        po = psum.tile([128, SCW], dt.float32, tag="big")
                        for h in range(HPC):
                            nc.tensor.matmul(
                                po[:, :],
                                wo[:, h, ec * 128:(ec + 1) * 128],
                                zT[:, h, :],
                                start=(h == 0), stop=(h == HPC - 1),
                            )
                        ob = work.tile([128, SCW], dt.float32, tag="ob")
                        nc.vector.tensor_copy(ob[:, :], po[:, :])
                        nc.sync.dma_start(
                            oT_d[ec * 128:(ec + 1) * 128, ssl], ob[:, :]
                        )

    nc.compile()
    names = {
        "xT": xT_d.name, "wq": wq_d.name, "wk": wk_d.name, "wv": wv_d.name,
        "wo": wo_d.name, "bq": bq_d.name, "bk": bk_d.name, "bv": bv_d.name,
        "oT": oT_d.name,
    }
    return nc, names


def get_program():
    global _PROG
    if _PROG is None:
        _PROG = _build_program()
    return _PROG


def make_in_maps(x, Wq, bq, Wk, bk, Wv, bv, Wo, names):
    """Host-side sharding: per-core input dict (bf16 casts + layout prep)."""
    in_maps = []
    for c in range(N_CORES):
        b, g = divmod(c, 2)
        hsl = slice(g * HPC, (g + 1) * HPC)
        xT_c = np.ascontiguousarray(x[b].T).astype(BF16)                 # [E, S]
        wq_c = np.ascontiguousarray(
            Wq[hsl].transpose(1, 0, 2).reshape(E, HPC * Dh)).astype(BF16)
        wk_c = np.ascontiguousarray(
            Wk[hsl].transpose(1, 0, 2).reshape(E, HPC * Dh)).astype(BF16)
        wv_c = np.ascontiguousarray(
            Wv[hsl].transpose(1, 0, 2).reshape(E, HPC * Dh)).astype(BF16)
        # Wo rows for this head group, packed [Dh, HPC, E] (head on free axis)
        wo_c = np.ascontiguousarray(
            Wo[g * HPC * Dh:(g + 1) * HPC * Dh].reshape(HPC, Dh, E)
            .transpose(1, 0, 2)).astype(BF16)
        bq_c = np.ascontiguousarray(bq[hsl].reshape(MP, 128).T).astype(np.float32)
        bk_c = np.ascontiguousarray(bk[hsl].reshape(MP, 128).T).astype(np.float32)
        bv_c = bv[hsl].reshape(1, HPC * Dh).astype(BF16)
        in_maps.append({
            names["xT"]: xT_c, names["wq"]: wq_c, names["wk"]: wk_c,
            names["wv"]: wv_c, names["wo"]: wo_c, names["bq"]: bq_c,
            names["bk"]: bk_c, names["bv"]: bv_c,
        })
    return in_maps


def combine_outputs(results, bo, names):
    """Host-side unshard: sum head-group partials, transpose, add bo."""
    out = np.empty((B, S, E), np.float32)
    for b in range(B):
        oT = results[2 * b][names["oT"]] + results[2 * b + 1][names["oT"]]
        out[b] = oT.T + bo
    return out


def kernel(x, Wq, bq, Wk, bk, Wv, bv, Wo, bo):
    from concourse.bass_utils import run_bass_kernel_spmd

    nc, names = get_program()
    in_maps = make_in_maps(
        np.asarray(x), np.asarray(Wq), np.asarray(bq), np.asarray(Wk),
        np.asarray(bk), np.asarray(Wv), np.asarray(bv), np.asarray(Wo), names,
    )
    res = run_bass_kernel_spmd(nc, in_maps, core_ids=list(range(N_CORES)))
    return combine_outputs(res.results, np.asarray(bo, np.float32), names)


# revision 12
# speedup vs baseline: 5.9326x; 1.1191x over previous
"""Trainium2 Bass kernel: 16-head MHA (B=4, S=2048, E=1024, Dh=64), 8 cores.

Sharding: core c handles batch b = c//2 and head-group g = c%2 (8 heads).
Each core computes its 8 heads' attention plus the partial output
projection in transposed layout oT[e, s]; the host sums the two
head-group partials per batch, transposes, and adds bo.

Per-core dataflow (all matmuls bf16 with fp32 PSUM accumulation):
  qT/kT[d, s]  = Wq/Wk.T @ xT          (per head-pair, d stacked 2x64)
  v[t, hd]     = xT.T @ Wv + ones.T@bv (natural layout, + ones col for colsum)
  scoresT[t,s] = kT.T @ qT   (row-tiled: 2 heads in rows 0-63 / 64-127)
  expT         = exp(0.125 * scoresT)  (ScalarE, cast to bf16)
  zT_un[d,s],colsum[s] = v_aug.T @ expT  (M=65: row 64 = colsum)
  zT           = zT_un * bcast(1/colsum) (bcast via K=1 matmul)
  oT[e, s]    += Wo_h.T @ zT_h  (accumulated over the core's 8 heads)
"""

import numpy as np
import ml_dtypes

B, S, E = 4, 2048, 1024
H, Dh = 16, 64
N_CORES = 8
HPC = 8          # heads per core
MP = 4           # head-pairs per core
SC, SCW = 4, 512  # s-chunks
TC, TCW = 16, 128  # t-chunks
KE = 8           # k-tiles over E
ECN = 8          # e-chunks of 128 (outT partition tiles)

BF16 = ml_dtypes.bfloat16

_PROG = None


def _build_program(repeats=None, timing=False):
    """Emit the Bass/Tile program. Returns (nc, names_dict).

    repeats: if set, wrap the whole body in a For_i loop (for marginal
    per-iteration HW timing; not used by the graded kernel() path).
    timing: demote the real output to internal DRAM and expose a tiny
    dummy output instead, so timing calls don't pay output transfers.
    """
    from contextlib import ExitStack

    import concourse.mybir as mybir
    import concourse.tile as tile
    from concourse import bacc

    dt = mybir.dt
    AF = mybir.ActivationFunctionType
    OP = mybir.AluOpType

    nc = bacc.Bacc(None, target_bir_lowering=False, debug=False)
    with tile.TileContext(nc) as tc:
        with tc.tile_pool(name="dram", bufs=1, space="DRAM") as dram:
            xT_d = dram.tile([E, S], dt.bfloat16, kind="ExternalInput")
            wq_d = dram.tile([E, HPC * Dh], dt.bfloat16, kind="ExternalInput")
            wk_d = dram.tile([E, HPC * Dh], dt.bfloat16, kind="ExternalInput")
            wv_d = dram.tile([E, HPC * Dh], dt.bfloat16, kind="ExternalInput")
            wo_d = dram.tile([Dh, HPC, E], dt.bfloat16, kind="ExternalInput")
            bq_d = dram.tile([128, MP], dt.float32, kind="ExternalInput")
            bk_d = dram.tile([128, MP], dt.float32, kind="ExternalInput")
            bv_d = dram.tile([1, HPC * Dh], dt.bfloat16, kind="ExternalInput")
            if timing:
                oT_d = dram.tile([E, S], dt.float32, kind="Internal")
                dummy_d = dram.tile([1, 4], dt.float32, kind="ExternalOutput")
            else:
                oT_d = dram.tile([E, S], dt.float32, kind="ExternalOutput")
                dummy_d = None

            with (
                tc.tile_pool(name="const", bufs=1) as const,
                tc.tile_pool(name="work", bufs=2) as work,
                tc.tile_pool(name="psum_sT", bufs=2, space="PSUM") as psum_sT,
                tc.tile_pool(name="psum_z", bufs=4, space="PSUM") as psum_z,
                ExitStack() as _es,
            ):
                if repeats is not None:
                    _es.enter_context(tc.For_i(
                        0, repeats, 1,
                        hint_engines=(
                            mybir.EngineType.PE, mybir.EngineType.Activation,
                            mybir.EngineType.DVE, mybir.EngineType.SP,
                            mybir.EngineType.Pool,
                        ),
                    ))
                # ---- persistent SBUF ----
                xT = const.tile([128, KE, S], dt.bfloat16)
                wq = const.tile([128, KE, HPC * Dh], dt.bfloat16)
                wk = const.tile([128, KE, HPC * Dh], dt.bfloat16)
                wv = const.tile([128, KE, HPC * Dh], dt.bfloat16)
                wo = const.tile([Dh, HPC, E], dt.bfloat16)
                bq = const.tile([128, MP], dt.float32)
                bk = const.tile([128, MP], dt.float32)
                bv = const.tile([1, HPC * Dh], dt.bfloat16)
                ones1 = const.tile([1, 128], dt.bfloat16)
                qT2 = const.tile([128, MP, S], dt.bfloat16)
                kT2 = const.tile([128, MP, S], dt.bfloat16)
                v_sb = const.tile([128, TC, HPC, Dh + 1], dt.bfloat16)

                # ---- input DMAs ----
                nc.sync.dma_start(xT[:, :, :], xT_d[:].rearrange("(a p) c -> p a c", p=128))
                nc.sync.dma_start(wq[:, :, :], wq_d[:].rearrange("(a p) c -> p a c", p=128))
                nc.sync.dma_start(wk[:, :, :], wk_d[:].rearrange("(a p) c -> p a c", p=128))
                nc.sync.dma_start(wv[:, :, :], wv_d[:].rearrange("(a p) c -> p a c", p=128))
                nc.sync.dma_start(wo[:, :, :], wo_d[:])
                nc.sync.dma_start(bq[:, :], bq_d[:])
                nc.sync.dma_start(bk[:, :], bk_d[:])
                nc.sync.dma_start(bv[:, :], bv_d[:])
                nc.vector.memset(ones1[:, :], 1.0)
                nc.vector.memset(v_sb[:, :, :, :], 1.0)
                if dummy_d is not None:
                    dum = const.tile([1, 4], dt.float32)
                    nc.vector.memset(dum[:, :], 1.0)
                    nc.sync.dma_start(dummy_d[:, :], dum[:, :])

                # ---- projections: kT2 (layout [d(2 heads), s]) + v, all s ----
                def proj_qk(w_sb, b_sb, dst, m, sc):
                    ssl = slice(sc * SCW, (sc + 1) * SCW)
                    p = psum_z.tile([128, SCW], dt.float32, tag="z")
                    for k in range(KE):
                        nc.tensor.matmul(
                            p[:, :],
                            w_sb[:, k, m * 128:(m + 1) * 128],
                            xT[:, k, ssl],
                            start=(k == 0), stop=(k == KE - 1),
                        )
                    nc.vector.tensor_scalar_add(
                        dst[:, m, ssl], p[:, :], b_sb[:, m:m + 1]
                    )

                for m in range(MP):
                    for sc in range(SC):
                        proj_qk(wk, bk, kT2, m, sc)

                # v projection (natural layout [t, hd] + bias + ones col)
                for t in range(TC):
                    tsl = slice(t * TCW, (t + 1) * TCW)
                    p = psum_z.tile([128, HPC * Dh], dt.float32, tag="z")
                    for k in range(KE):
                        nc.tensor.matmul(
                            p[:, :], xT[:, k, tsl], wv[:, k, :],
                            start=(k == 0), stop=False,
                        )
                    nc.tensor.matmul(
                        p[:, :], ones1[0:1, :], bv[0:1, :], start=False, stop=True,
                    )
                    nc.vector.tensor_copy(
                        v_sb[:, t, :, 0:Dh],
                        p[:, :].rearrange("p (h c) -> p h c", c=Dh),
                    )

                # ---- attention + output projection, per s-chunk ----
                def emit_scores_exp(h, sc):
                    m, hh = divmod(h, 2)
                    hoff = hh * Dh
                    ssl = slice(sc * SCW, (sc + 1) * SCW)
                    eT = work.tile([128, TC, SCW], dt.bfloat16, tag="expT")
                    for t2 in range(TC // 2):
                        pst = psum_sT.tile([128, 2 * SCW], dt.float32, tag="sT")
                        for j in range(2):
                            t = 2 * t2 + j
                            nc.tensor.matmul(
                                pst[:, j * SCW:(j + 1) * SCW],
                                kT2[hoff:hoff + Dh, m, t * TCW:(t + 1) * TCW],
                                qT2[hoff:hoff + Dh, m, ssl],
                                start=True, stop=True,
                                tile_position=(hoff, 0),
                            )
                        nc.scalar.activation(
                            eT[:, 2 * t2:2 * t2 + 2, :], pst[:, :],
                            AF.Exp, scale=0.125,
                        )
                    return eT

                def emit_av_norm(h, eT, zT):
                    pz = psum_z.tile([Dh + 1, SCW], dt.float32, tag="z")
                    for t in range(TC):
                        nc.tensor.matmul(
                            pz[:, :], v_sb[:, t, h, :], eT[:, t, :],
                            start=(t == 0), stop=(t == TC - 1),
                        )
                    cs = work.tile([1, SCW], dt.bfloat16, tag="cs")
                    nc.vector.tensor_copy(cs[0:1, :], pz[Dh:Dh + 1, :])
                    pbc = psum_z.tile([Dh, SCW], dt.float32, tag="z")
                    nc.tensor.matmul(
                        pbc[:, :], ones1[0:1, 0:Dh], cs[0:1, :],
                        start=True, stop=True,
                    )
                    bch = work.tile([Dh, SCW], dt.float32, tag="bch")
                    nc.vector.reciprocal(bch[:, :], pbc[:, :])
                    nc.vector.tensor_tensor(
                        zT[:, h, :], pz[0:Dh, :], bch[:, :], OP.mult
                    )

                for sc in range(SC):
                    ssl = slice(sc * SCW, (sc + 1) * SCW)
                    for m in range(MP):
                        proj_qk(wq, bq, qT2, m, sc)
                    zT = work.tile([Dh, HPC, SCW], dt.bfloat16, tag="zT")
                    prev = None
                    for h in range(HPC):
                        eT = emit_scores_exp(h, sc)
                        if prev is not None:
                            emit_av_norm(prev[0], prev[1], zT)
                        prev = (h, eT)
                    emit_av_norm(prev[0], prev[1], zT)
                    for ec in range(ECN):
                        po = psum_z.tile([128, SCW], dt.float32, tag="z")
                        for h in range(HPC):
                            nc.tensor.matmul(
                                po[:, :],
                                wo[:, h, ec * 128:(ec + 1) * 128],
                                zT[:, h, :],
                                start=(h == 0), stop=(h == HPC - 1),
                            )
                        ob = work.tile([128, SCW], dt.float32, tag="ob")
                        nc.vector.tensor_copy(ob[:, :], po[:, :])
                        nc.sync.dma_start(
                            oT_d[ec * 128:(ec + 1) * 128, ssl], ob[:, :]
                        )

    nc.compile()
    names = {
        "xT": xT_d.name, "wq": wq_d.name, "wk": wk_d.name, "wv": wv_d.name,
        "wo": wo_d.name, "bq": bq_d.name, "bk": bk_d.name, "bv": bv_d.name,
        "oT": oT_d.name,
    }
    return nc, names


def get_program():
    global _PROG
    if _PROG is None:
        _PROG = _build_program()
    return _PROG


def make_in_maps(x, Wq, bq, Wk, bk, Wv, bv, Wo, names):
    """Host-side sharding: per-core input dict (bf16 casts + layout prep)."""
    in_maps = []
    for c in range(N_CORES):
        b, g = divmod(c, 2)
        hsl = slice(g * HPC, (g + 1) * HPC)
        xT_c = np.ascontiguousarray(x[b].T).astype(BF16)                 # [E, S]
        wq_c = np.ascontiguousarray(
            Wq[hsl].transpose(1, 0, 2).reshape(E, HPC * Dh)).astype(BF16)
        wk_c = np.ascontiguousarray(
            Wk[hsl].transpose(1, 0, 2).reshape(E, HPC * Dh)).astype(BF16)
        wv_c = np.ascontiguousarray(
            Wv[hsl].transpose(1, 0, 2).reshape(E, HPC * Dh)).astype(BF16)
        # Wo rows for this head group, packed [Dh, HPC, E] (head on free axis)
        wo_c = np.ascontiguousarray(
            Wo[g * HPC * Dh:(g + 1) * HPC * Dh].reshape(HPC, Dh, E)
            .transpose(1, 0, 2)).astype(BF16)
        bq_c = np.ascontiguousarray(bq[hsl].reshape(MP, 128).T).astype(np.float32)
        bk_c = np.ascontiguousarray(bk[hsl].reshape(MP, 128).T).astype(np.float32)
        bv_c = bv[hsl].reshape(1, HPC * Dh).astype(BF16)
        in_maps.append({
            names["xT"]: xT_c, names["wq"]: wq_c, names["wk"]: wk_c,
            names["wv"]: wv_c, names["wo"]: wo_c, names["bq"]: bq_c,
            names["bk"]: bk_c, names["bv"]: bv_c,
        })
    return in_maps


def combine_outputs(results, bo, names):
    """Host-side unshard: sum head-group partials, transpose, add bo."""
    out = np.empty((B, S, E), np.float32)
    for b in range(B):
        oT = results[2 * b][names["oT"]] + results[2 * b + 1][names["oT"]]
        out[b] = oT.T + bo
    return out


def kernel(x, Wq, bq, Wk, bk, Wv, bv, Wo, bo):
    from concourse.bass_utils import run_bass_kernel_spmd

    nc, names = get_program()
    in_maps = make_in_maps(
        np.asarray(x), np.asarray(Wq), np.asarray(bq), np.asarray(Wk),
        np.asarray(bk), np.asarray(Wv), np.asarray(bv), np.asarray(Wo), names,
    )
    res = run_bass_kernel_spmd(nc, in_maps, core_ids=list(range(N_CORES)))
    return combine_outputs(res.results, np.asarray(bo, np.float32), names)


# revision 14
# speedup vs baseline: 6.8597x; 1.1563x over previous
"""Trainium2 Bass kernel: 16-head MHA (B=4, S=2048, E=1024, Dh=64), 8 cores.

Sharding: core c handles batch b = c//2 and head-group g = c%2 (8 heads).
Each core computes its 8 heads' attention plus the partial output
projection in transposed layout oT[e, s]; the host sums the two
head-group partials per batch, transposes, and adds bo.

Per-core dataflow (all matmuls bf16 with fp32 PSUM accumulation):
  qT/kT[d, s]  = Wq/Wk.T @ xT          (per head-pair, d stacked 2x64)
  v[t, hd]     = xT.T @ Wv + ones.T@bv (natural layout, + ones col for colsum)
  scoresT[t,s] = kT.T @ qT   (row-tiled: 2 heads in rows 0-63 / 64-127)
  expT         = exp(0.125 * scoresT)  (ScalarE, cast to bf16)
  zT_un[d,s],colsum[s] = v_aug.T @ expT  (M=65: row 64 = colsum)
  zT           = zT_un * bcast(1/colsum) (bcast via K=1 matmul)
  oT[e, s]    += Wo_h.T @ zT_h  (accumulated over the core's 8 heads)
"""

import numpy as np
import ml_dtypes

B, S, E = 4, 2048, 1024
H, Dh = 16, 64
N_CORES = 8
HPC = 8          # heads per core
MP = 4           # head-pairs per core
SC, SCW = 4, 512  # s-chunks
TC, TCW = 16, 128  # t-chunks
KE = 8           # k-tiles over E
ECN = 8          # e-chunks of 128 (outT partition tiles)

BF16 = ml_dtypes.bfloat16

_PROG = None


def _build_program(repeats=None, timing=False, parts=3):
    """Emit the Bass/Tile program. Returns (nc, names_dict).

    repeats: if set, wrap the whole body in a For_i loop (for marginal
    per-iteration HW timing; not used by the graded kernel() path).
    timing: demote the real output to internal DRAM and expose a tiny
    dummy output instead, so timing calls don't pay output transfers.
    parts: 1 = projections only, 2 = + attention, 3 = full (default).
    """
    from contextlib import ExitStack

    import concourse.mybir as mybir
    import concourse.tile as tile
    from concourse import bacc

    dt = mybir.dt
    AF = mybir.ActivationFunctionType
    OP = mybir.AluOpType

    nc = bacc.Bacc(None, target_bir_lowering=False, debug=False)
    with tile.TileContext(nc) as tc:
        with tc.tile_pool(name="dram", bufs=1, space="DRAM") as dram:
            xT_d = dram.tile([E, S], dt.bfloat16, kind="ExternalInput")
            wq_d = dram.tile([E, HPC * Dh], dt.bfloat16, kind="ExternalInput")
            wk_d = dram.tile([E, HPC * Dh], dt.bfloat16, kind="ExternalInput")
            wv_d = dram.tile([E, HPC * Dh], dt.bfloat16, kind="ExternalInput")
            wo_d = dram.tile([Dh, HPC, E], dt.bfloat16, kind="ExternalInput")
            bq_d = dram.tile([128, MP], dt.float32, kind="ExternalInput")
            bk_d = dram.tile([128, MP], dt.float32, kind="ExternalInput")
            bv_d = dram.tile([1, HPC * Dh], dt.bfloat16, kind="ExternalInput")
            if timing:
                oT_d = dram.tile([E, S], dt.float32, kind="Internal")
                dummy_d = dram.tile([1, 4], dt.float32, kind="ExternalOutput")
            else:
                oT_d = dram.tile([E, S], dt.float32, kind="ExternalOutput")
                dummy_d = None

            with (
                tc.tile_pool(name="const", bufs=1) as const,
                tc.tile_pool(name="work", bufs=2) as work,
                tc.tile_pool(name="psum_sT", bufs=2, space="PSUM") as psum_sT,
                tc.tile_pool(name="psum_z", bufs=4, space="PSUM") as psum_z,
                ExitStack() as _es,
            ):
                if repeats is not None:
                    _es.enter_context(tc.For_i(
                        0, repeats, 1,
                        hint_engines=(
                            mybir.EngineType.PE, mybir.EngineType.Activation,
                            mybir.EngineType.DVE, mybir.EngineType.SP,
                            mybir.EngineType.Pool,
                        ),
                    ))
                # ---- persistent SBUF ----
                xT = const.tile([128, KE, S], dt.bfloat16)
                wq = const.tile([128, KE, HPC * Dh], dt.bfloat16)
                wk = const.tile([128, KE, HPC * Dh], dt.bfloat16)
                wv = const.tile([128, KE, HPC * Dh], dt.bfloat16)
                wo = const.tile([Dh, HPC, E], dt.bfloat16)
                bq = const.tile([128, MP], dt.float32)
                bk = const.tile([128, MP], dt.float32)
                bv = const.tile([1, HPC * Dh], dt.bfloat16)
                ones1 = const.tile([1, 128], dt.bfloat16)
                qT2 = const.tile([128, MP, S], dt.bfloat16)
                kT2 = const.tile([128, MP, S], dt.bfloat16)
                v_sb = const.tile([128, TC, HPC, Dh + 1], dt.bfloat16)

                # ---- input DMAs ----
                nc.sync.dma_start(xT[:, :, :], xT_d[:].rearrange("(a p) c -> p a c", p=128))
                nc.sync.dma_start(wq[:, :, :], wq_d[:].rearrange("(a p) c -> p a c", p=128))
                nc.sync.dma_start(wk[:, :, :], wk_d[:].rearrange("(a p) c -> p a c", p=128))
                nc.sync.dma_start(wv[:, :, :], wv_d[:].rearrange("(a p) c -> p a c", p=128))
                nc.sync.dma_start(wo[:, :, :], wo_d[:])
                nc.sync.dma_start(bq[:, :], bq_d[:])
                nc.sync.dma_start(bk[:, :], bk_d[:])
                nc.sync.dma_start(bv[:, :], bv_d[:])
                nc.vector.memset(ones1[:, :], 1.0)
                nc.vector.memset(v_sb[:, :, :, :], 1.0)
                if dummy_d is not None:
                    dum = const.tile([1, 4], dt.float32)
                    nc.vector.memset(dum[:, :], 1.0)
                    nc.sync.dma_start(dummy_d[:, :], dum[:, :])

                # ---- projections: kT2 (layout [d(2 heads), s]) + v, all s ----
                def proj_qk(w_sb, b_sb, dst, m, sc):
                    ssl = slice(sc * SCW, (sc + 1) * SCW)
                    p = psum_z.tile([128, SCW], dt.float32, tag="z")
                    for k in range(KE):
                        nc.tensor.matmul(
                            p[:, :],
                            w_sb[:, k, m * 128:(m + 1) * 128],
                            xT[:, k, ssl],
                            start=(k == 0), stop=(k == KE - 1),
                        )
                    nc.vector.tensor_scalar_add(
                        dst[:, m, ssl], p[:, :], b_sb[:, m:m + 1]
                    )

                for m in range(MP):
                    for sc in range(SC):
                        proj_qk(wk, bk, kT2, m, sc)

                # v projection (natural layout [t, hd] + bias + ones col)
                for t in range(TC):
                    tsl = slice(t * TCW, (t + 1) * TCW)
                    p = psum_z.tile([128, HPC * Dh], dt.float32, tag="z")
                    for k in range(KE):
                        nc.tensor.matmul(
                            p[:, :], xT[:, k, tsl], wv[:, k, :],
                            start=(k == 0), stop=False,
                        )
                    nc.tensor.matmul(
                        p[:, :], ones1[0:1, :], bv[0:1, :], start=False, stop=True,
                    )
                    nc.vector.tensor_copy(
                        v_sb[:, t, :, 0:Dh],
                        p[:, :].rearrange("p (h c) -> p h c", c=Dh),
                    )

                # ---- attention + output projection, per s-chunk ----
                def emit_scores_exp(h, sc):
                    m, hh = divmod(h, 2)
                    hoff = hh * Dh
                    ssl = slice(sc * SCW, (sc + 1) * SCW)
                    eT = work.tile([128, TC, SCW], dt.bfloat16, tag="expT")
                    for t2 in range(TC // 2):
                        pst = psum_sT.tile([128, 2 * SCW], dt.float32, tag="sT")
                        for j in range(2):
                            t = 2 * t2 + j
                            nc.tensor.matmul(
                                pst[:, j * SCW:(j + 1) * SCW],
                                kT2[hoff:hoff + Dh, m, t * TCW:(t + 1) * TCW],
                                qT2[hoff:hoff + Dh, m, ssl],
                                start=True, stop=True,
                                tile_position=(hoff, 0),
                            )
                        nc.scalar.activation(
                            eT[:, 2 * t2:2 * t2 + 2, :], pst[:, :],
                            AF.Exp, scale=0.125,
                        )
                    return eT

                def emit_av_norm(h, eT, zT):
                    pz = psum_z.tile([Dh + 1, SCW], dt.float32, tag="z")
                    for t in range(TC):
                        nc.tensor.matmul(
                            pz[:, :], v_sb[:, t, h, :], eT[:, t, :],
                            start=(t == 0), stop=(t == TC - 1),
                        )
                    cs = work.tile([1, SCW], dt.bfloat16, tag="cs")
                    nc.vector.tensor_copy(cs[0:1, :], pz[Dh:Dh + 1, :])
                    pbc = psum_z.tile([Dh, SCW], dt.float32, tag="z")
                    nc.tensor.matmul(
                        pbc[:, :], ones1[0:1, 0:Dh], cs[0:1, :],
                        start=True, stop=True,
                    )
                    bch = work.tile([Dh, SCW], dt.float32, tag="bch")
                    nc.vector.reciprocal(bch[:, :], pbc[:, :])
                    nc.vector.tensor_tensor(
                        zT[:, h, :], pz[0:Dh, :], bch[:, :], OP.mult
                    )

                for sc in range(SC):
                    ssl = slice(sc * SCW, (sc + 1) * SCW)
                    for m in range(MP):
                        proj_qk(wq, bq, qT2, m, sc)
                    if parts < 2:
                        continue
                    zT = work.tile([Dh, HPC, SCW], dt.bfloat16, tag="zT")
                    prev = None
                    for h in range(HPC):
                        eT = emit_scores_exp(h, sc)
                        if prev is not None:
                            emit_av_norm(prev[0], prev[1], zT)
                        prev = (h, eT)
                    emit_av_norm(prev[0], prev[1], zT)
                    if parts < 3:
                        continue
                    for ec in range(ECN):
                        po = psum_z.tile([128, SCW], dt.float32, tag="z")
                        for h in range(HPC):
                            nc.tensor.matmul(
                                po[:, :],
                                wo[:, h, ec * 128:(ec + 1) * 128],
                                zT[:, h, :],
                                start=(h == 0), stop=(h == HPC - 1),
                            )
                        ob = work.tile([128, SCW], dt.float32, tag="ob")
                        nc.vector.tensor_copy(ob[:, :], po[:, :])
                        nc.sync.dma_start(
                            oT_d[ec * 128:(ec + 1) * 128, ssl], ob[:, :]
                        )

    nc.compile()
    names = {
        "xT": xT_d.name, "wq": wq_d.name, "wk": wk_d.name, "wv": wv_d.name,
        "wo": wo_d.name, "bq": bq_d.name, "bk": bk_d.name, "bv": bv_d.name,
        "oT": oT_d.name,
    }
    return nc, names


def get_program():
    global _PROG
    if _PROG is None:
        _PROG = _build_program()
    return _PROG


def make_in_maps(x, Wq, bq, Wk, bk, Wv, bv, Wo, names):
    """Host-side sharding: per-core input dict (bf16 casts + layout prep)."""
    in_maps = []
    for c in range(N_CORES):
        b, g = divmod(c, 2)
        hsl = slice(g * HPC, (g + 1) * HPC)
        xT_c = np.ascontiguousarray(x[b].T).astype(BF16)                 # [E, S]
        wq_c = np.ascontiguousarray(
            Wq[hsl].transpose(1, 0, 2).reshape(E, HPC * Dh)).astype(BF16)
        wk_c = np.ascontiguousarray(
            Wk[hsl].transpose(1, 0, 2).reshape(E, HPC * Dh)).astype(BF16)
        wv_c = np.ascontiguousarray(
            Wv[hsl].transpose(1, 0, 2).reshape(E, HPC * Dh)).astype(BF16)
        # Wo rows for this head group, packed [Dh, HPC, E] (head on free axis)
        wo_c = np.ascontiguousarray(
            Wo[g * HPC * Dh:(g + 1) * HPC * Dh].reshape(HPC, Dh, E)
            .transpose(1, 0, 2)).astype(BF16)
        bq_c = np.ascontiguousarray(bq[hsl].reshape(MP, 128).T).astype(np.float32)
        bk_c = np.ascontiguousarray(bk[hsl].reshape(MP, 128).T).astype(np.float32)
        bv_c = bv[hsl].reshape(1, HPC * Dh).astype(BF16)
        in_maps.append({
            names["xT"]: xT_c, names["wq"]: wq_c, names["wk"]: wk_c,
            names["wv"]: wv_c, names["wo"]: wo_c, names["bq"]: bq_c,
            names["bk"]: bk_c, names["bv"]: bv_c,
        })
    return in_maps


def combine_outputs(results, bo, names):
    """Host-side unshard: sum head-group partials, transpose, add bo."""
    out = np.empty((B, S, E), np.float32)
    for b in range(B):
        oT = results[2 * b][names["oT"]] + results[2 * b + 1][names["oT"]]
        out[b] = oT.T + bo
    return out


def kernel(x, Wq, bq, Wk, bk, Wv, bv, Wo, bo):
    from concourse.bass_utils import run_bass_kernel_spmd

    nc, names = get_program()
    in_maps = make_in_maps(
        np.asarray(x), np.asarray(Wq), np.asarray(bq), np.asarray(Wk),
        np.asarray(bk), np.asarray(Wv), np.asarray(bv), np.asarray(Wo), names,
    )
    res = run_bass_kernel_spmd(nc, in_maps, core_ids=list(range(N_CORES)))
    return combine_outputs(res.results, np.asarray(bo, np.float32), names)


# revision 15
# speedup vs baseline: 25.5934x; 3.7310x over previous
"""Trainium2 Bass kernel: 16-head MHA (B=4, S=2048, E=1024, Dh=64), 8 cores.

Sharding: core c handles batch b = c//2 and head-group g = c%2 (8 heads).
Each core computes its 8 heads' attention plus the partial output
projection in transposed layout oT[e, s]; the host sums the two
head-group partials per batch, transposes, and adds bo.

Per-core dataflow (all matmuls bf16 with fp32 PSUM accumulation):
  qT/kT[d, s]  = Wq/Wk.T @ xT          (per head-pair, d stacked 2x64)
  v[t, hd]     = xT.T @ Wv + ones.T@bv (natural layout, + ones col for colsum)
  scoresT[t,s] = kT.T @ qT   (row-tiled: 2 heads in rows 0-63 / 64-127)
  expT         = exp(0.125 * scoresT)  (ScalarE, cast to bf16)
  zT_un[d,s],colsum[s] = v_aug.T @ expT  (M=65: row 64 = colsum)
  zT           = zT_un * bcast(1/colsum) (bcast via K=1 matmul)
  oT[e, s]    += Wo_h.T @ zT_h  (accumulated over the core's 8 heads)
"""

import numpy as np
import ml_dtypes

B, S, E = 4, 2048, 1024
H, Dh = 16, 64
N_CORES = 8
HPC = 8          # heads per core
MP = 4           # head-pairs per core
SC, SCW = 4, 512  # s-chunks
TC, TCW = 16, 128  # t-chunks
KE = 8           # k-tiles over E
ECN = 8          # e-chunks of 128 (outT partition tiles)

BF16 = ml_dtypes.bfloat16

_PROG = None


def _build_program(repeats=None, timing=False, parts=3):
    """Emit the Bass/Tile program. Returns (nc, names_dict).

    repeats: if set, wrap the whole body in a For_i loop (for marginal
    per-iteration HW timing; not used by the graded kernel() path).
    timing: demote the real output to internal DRAM and expose a tiny
    dummy output instead, so timing calls don't pay output transfers.
    parts: 1 = projections only, 2 = + attention, 3 = full (default).
    """
    from contextlib import ExitStack

    import concourse.mybir as mybir
    import concourse.tile as tile
    from concourse import bacc

    dt = mybir.dt
    AF = mybir.ActivationFunctionType
    OP = mybir.AluOpType

    nc = bacc.Bacc(None, target_bir_lowering=False, debug=False)
    with tile.TileContext(nc) as tc:
        with tc.tile_pool(name="dram", bufs=1, space="DRAM") as dram:
            xT_d = dram.tile([E, S], dt.bfloat16, kind="ExternalInput")
            wq_d = dram.tile([E, HPC * Dh], dt.bfloat16, kind="ExternalInput")
            wk_d = dram.tile([E, HPC * Dh], dt.bfloat16, kind="ExternalInput")
            wv_d = dram.tile([E, HPC * Dh], dt.bfloat16, kind="ExternalInput")
            wo_d = dram.tile([Dh, HPC, E], dt.bfloat16, kind="ExternalInput")
            bq_d = dram.tile([128, MP], dt.float32, kind="ExternalInput")
            bk_d = dram.tile([128, MP], dt.float32, kind="ExternalInput")
            bv_d = dram.tile([1, HPC * Dh], dt.bfloat16, kind="ExternalInput")
            if timing:
                oT_d = dram.tile([E, S], dt.float32, kind="Internal")
                dummy_d = dram.tile([1, 4], dt.float32, kind="ExternalOutput")
            else:
                oT_d = dram.tile([E, S], dt.float32, kind="ExternalOutput")
                dummy_d = None

            with (
                tc.tile_pool(name="const", bufs=1) as const,
                tc.tile_pool(name="work", bufs=2) as work,
                tc.tile_pool(name="psum_sT", bufs=2, space="PSUM") as psum_sT,
                tc.tile_pool(name="psum_z", bufs=4, space="PSUM") as psum_z,
                ExitStack() as _es,
            ):
                if repeats is not None:
                    _es.enter_context(tc.For_i(
                        0, repeats, 1,
                        hint_engines=(
                            mybir.EngineType.PE, mybir.EngineType.Activation,
                            mybir.EngineType.DVE, mybir.EngineType.SP,
                            mybir.EngineType.Pool,
                        ),
                    ))
                # ---- persistent SBUF ----
                xT = const.tile([128, KE, S], dt.bfloat16)
                wq = const.tile([128, KE, HPC * Dh], dt.bfloat16)
                wk = const.tile([128, KE, HPC * Dh], dt.bfloat16)
                wv = const.tile([128, KE, HPC * Dh], dt.bfloat16)
                wo = const.tile([Dh, HPC, E], dt.bfloat16)
                bq = const.tile([128, MP], dt.float32)
                bk = const.tile([128, MP], dt.float32)
                bv = const.tile([1, HPC * Dh], dt.bfloat16)
                ones1 = const.tile([1, 128], dt.bfloat16)
                qT2 = const.tile([128, MP, S], dt.bfloat16)
                kT2 = const.tile([128, MP, S], dt.bfloat16)
                v_sb = const.tile([128, TC, HPC, Dh + 1], dt.bfloat16)

                # ---- input DMAs ----
                nc.sync.dma_start(xT[:, :, :], xT_d[:].rearrange("(a p) c -> p a c", p=128))
                nc.sync.dma_start(wq[:, :, :], wq_d[:].rearrange("(a p) c -> p a c", p=128))
                nc.sync.dma_start(wk[:, :, :], wk_d[:].rearrange("(a p) c -> p a c", p=128))
                nc.sync.dma_start(wv[:, :, :], wv_d[:].rearrange("(a p) c -> p a c", p=128))
                nc.sync.dma_start(wo[:, :, :], wo_d[:])
                nc.sync.dma_start(bq[:, :], bq_d[:])
                nc.sync.dma_start(bk[:, :], bk_d[:])
                nc.sync.dma_start(bv[:, :], bv_d[:])
                nc.vector.memset(ones1[:, :], 1.0)
                nc.vector.memset(v_sb[:, :, :, :], 1.0)
                if dummy_d is not None:
                    dum = const.tile([1, 4], dt.float32)
                    nc.vector.memset(dum[:, :], 1.0)
                    nc.sync.dma_start(dummy_d[:, :], dum[:, :])

                # ---- projections: kT2 (layout [d(2 heads), s]) + v, all s ----
                def proj_qk(w_sb, b_sb, dst, m, sc):
                    ssl = slice(sc * SCW, (sc + 1) * SCW)
                    p = psum_z.tile([128, SCW], dt.float32, tag="z")
                    for k in range(KE):
                        nc.tensor.matmul(
                            p[:, :],
                            w_sb[:, k, m * 128:(m + 1) * 128],
                            xT[:, k, ssl],
                            start=(k == 0), stop=(k == KE - 1),
                        )
                    nc.vector.tensor_scalar_add(
                        dst[:, m, ssl], p[:, :], b_sb[:, m:m + 1]
                    )

                for m in range(MP):
                    for sc in range(SC):
                        proj_qk(wk, bk, kT2, m, sc)

                # v projection (natural layout [t, hd] + bias + ones col)
                for t in range(TC):
                    tsl = slice(t * TCW, (t + 1) * TCW)
                    p = psum_z.tile([128, HPC * Dh], dt.float32, tag="z")
                    for k in range(KE):
                        nc.tensor.matmul(
                            p[:, :], xT[:, k, tsl], wv[:, k, :],
                            start=(k == 0), stop=False,
                        )
                    nc.tensor.matmul(
                        p[:, :], ones1[0:1, :], bv[0:1, :], start=False, stop=True,
                    )
                    nc.vector.tensor_copy(
                        v_sb[:, t, :, 0:Dh],
                        p[:, :].rearrange("p (h c) -> p h c", c=Dh),
                    )

                # ---- attention + output projection, per s-chunk ----
                def emit_av_pair(h, eT, pz, t2):
                    for t in (2 * t2, 2 * t2 + 1):
                        nc.tensor.matmul(
                            pz[:, :], v_sb[:, t, h, :], eT[:, t, :],
                            start=(t == 0), stop=(t == TC - 1),
                        )

                def emit_norm(h, pz, zT):
                    cs = work.tile([1, SCW], dt.bfloat16, tag="cs")
                    nc.vector.tensor_copy(cs[0:1, :], pz[Dh:Dh + 1, :])
                    pbc = psum_z.tile([Dh, SCW], dt.float32, tag="z")
                    nc.tensor.matmul(
                        pbc[:, :], ones1[0:1, 0:Dh], cs[0:1, :],
                        start=True, stop=True,
                    )
                    bch = work.tile([Dh, SCW], dt.float32, tag="bch")
                    nc.vector.reciprocal(bch[:, :], pbc[:, :])
                    nc.vector.tensor_tensor(
                        zT[:, h, :], pz[0:Dh, :], bch[:, :], OP.mult
                    )

                def emit_head(h, sc, prev, zT):
                    """Scores+exp for head h, interleaved with AV of prev head.
                    prev = (h_prev, eT_prev) or None. Returns (h, eT)."""
                    ssl = slice(sc * SCW, (sc + 1) * SCW)
                    pz = None
                    if prev is not None:
                        pz = psum_z.tile([Dh + 1, SCW], dt.float32, tag="z")
                    if h is not None:
                        m, hh = divmod(h, 2)
                        hoff = hh * Dh
                        eT = work.tile([128, TC, SCW], dt.bfloat16, tag="expT")
                    else:
                        eT = None
                    for t2 in range(TC // 2):
                        if eT is not None:
                            pst = psum_sT.tile([128, 2 * SCW], dt.float32, tag="sT")
                            for j in range(2):
                                t = 2 * t2 + j
                                nc.tensor.matmul(
                                    pst[:, j * SCW:(j + 1) * SCW],
                                    kT2[hoff:hoff + Dh, m, t * TCW:(t + 1) * TCW],
                                    qT2[hoff:hoff + Dh, m, ssl],
                                    start=True, stop=True,
                                    tile_position=(hoff, 0),
                                )
                            nc.scalar.activation(
                                eT[:, 2 * t2:2 * t2 + 2, :], pst[:, :],
                                AF.Exp, scale=0.125,
                            )
                        if prev is not None:
                            emit_av_pair(prev[0], prev[1], pz, t2)
                    if prev is not None:
                        emit_norm(prev[0], pz, zT)
                    return (h, eT)

                for sc in range(SC):
                    ssl = slice(sc * SCW, (sc + 1) * SCW)
                    for m in range(MP):
                        proj_qk(wq, bq, qT2, m, sc)
                    if parts < 2:
                        continue
                    zT = work.tile([Dh, HPC, SCW], dt.bfloat16, tag="zT")
                    prev = None
                    for h in range(HPC):
                        prev = emit_head(h, sc, prev, zT)
                    emit_head(None, sc, prev, zT)
                    if parts < 3:
                        continue
                    for ec in range(ECN):
                        po = psum_z.tile([128, SCW], dt.float32, tag="z")
                        for h in range(HPC):
                            nc.tensor.matmul(
                                po[:, :],
                                wo[:, h, ec * 128:(ec + 1) * 128],
                                zT[:, h, :],
                                start=(h == 0), stop=(h == HPC - 1),
                            )
                        ob = work.tile([128, SCW], dt.float32, tag="ob")
                        nc.vector.tensor_copy(ob[:, :], po[:, :])
                        nc.sync.dma_start(
                            oT_d[ec * 128:(ec + 1) * 128, ssl], ob[:, :]
                        )

    nc.compile()
    names = {
        "xT": xT_d.name, "wq": wq_d.name, "wk": wk_d.name, "wv": wv_d.name,
        "wo": wo_d.name, "bq": bq_d.name, "bk": bk_d.name, "bv": bv_d.name,
        "oT": oT_d.name,
    }
    return nc, names


def get_program():
    global _PROG
    if _PROG is None:
        _PROG = _build_program()
    return _PROG


def make_in_maps(x, Wq, bq, Wk, bk, Wv, bv, Wo, names):
    """Host-side sharding: per-core input dict (bf16 casts + layout prep)."""
    in_maps = []
    for c in range(N_CORES):
        b, g = divmod(c, 2)
        hsl = slice(g * HPC, (g + 1) * HPC)
        xT_c = np.ascontiguousarray(x[b].T).astype(BF16)                 # [E, S]
        wq_c = np.ascontiguousarray(
            Wq[hsl].transpose(1, 0, 2).reshape(E, HPC * Dh)).astype(BF16)
        wk_c = np.ascontiguousarray(
            Wk[hsl].transpose(1, 0, 2).reshape(E, HPC * Dh)).astype(BF16)
        wv_c = np.ascontiguousarray(
            Wv[hsl].transpose(1, 0, 2).reshape(E, HPC * Dh)).astype(BF16)
        # Wo rows for this head group, packed [Dh, HPC, E] (head on free axis)
        wo_c = np.ascontiguousarray(
            Wo[g * HPC * Dh:(g + 1) * HPC * Dh].reshape(HPC, Dh, E)
            .transpose(1, 0, 2)).astype(BF16)
        bq_c = np.ascontiguousarray(bq[hsl].reshape(MP, 128).T).astype(np.float32)
        bk_c = np.ascontiguousarray(bk[hsl].reshape(MP, 128).T).astype(np.float32)
        bv_c = bv[hsl].reshape(1, HPC * Dh).astype(BF16)
        in_maps.append({
            names["xT"]: xT_c, names["wq"]: wq_c, names["wk"]: wk_c,
            names["wv"]: wv_c, names["wo"]: wo_c, names["bq"]: bq_c,
            names["bk"]: bk_c, names["bv"]: bv_c,
        })
    return in_maps


def combine_outputs(results, bo, names):
    """Host-side unshard: sum head-group partials, transpose, add bo."""
    out = np.empty((B, S, E), np.float32)
    for b in range(B):
        oT = results[2 * b][names["oT"]] + results[2 * b + 1][names["oT"]]
        out[b] = oT.T + bo
    return out


def kernel(x, Wq, bq, Wk, bk, Wv, bv, Wo, bo):
    from concourse.bass_utils import run_bass_kernel_spmd

    nc, names = get_program()
    in_maps = make_in_maps(
        np.asarray(x), np.asarray(Wq), np.asarray(bq), np.asarray(Wk),
        np.asarray(bk), np.asarray(Wv), np.asarray(bv), np.asarray(Wo), names,
    )
    res = run_bass_kernel_spmd(nc, in_maps, core_ids=list(range(N_CORES)))
    return combine_outputs(res.results, np.asarray(bo, np.float32), names)
